# revision 17
# baseline (speedup 1.0000x reference)
"""Trainium2 8-core kernel for batched attention + concat projection.

Reference computation (per batch b):
    scores = Q @ C^T                  [TQ, TC]
    A      = softmax(scores, axis=-1)
    mix    = A @ C                    [TQ, H]
    out    = tanh(concat([mix, Q]) @ W^T)   [TQ, H]

Distribution: pure data-parallel over batch (B=16 across 8 cores, 2
batches per core), W replicated. No collectives needed.

Per-core dataflow (activations kept in "transposed" [feature, token]
layout so every matmul contracts over the partition axis):
  - CT = C^T (f32r) and QT = Q^T built on-device via PE transposes.
  - scores S[q,k] = QT.T @ CT  (f32r matmuls, 1 col/cycle).
  - softmax over free axis k: DVE reduce_max(negate) -> ACT exp with
    per-partition bias, bf16 output (unnormalized, max ~= 1) and
    fp32 row-sum accumulator -> DVE reciprocal.
  - P^T via bf16 PE transposes, mix^T = C.T @ P^T in bf16.
  - normalization folded into the PV PSUM drain: multiply by a
    [128, sq] broadcast of 1/rowsum built once per super-iteration on
    the PE (transpose rcp to a row + ones outer-product matmul).
  - proj: out[q, :] = tanh(combT.T @ W^T) in bf16, W^T pre-transposed
    on host.

The P^T/PV/proj stages for super-iteration s are emitted one
super-iteration later (software pipelining) so the in-order TensorE
stream always has ready matmul work while the softmax chain of the
current tile runs on ACT/DVE.
"""

import numpy as np
import ml_dtypes

import concourse.bacc as bacc
import concourse.tile as tile
import concourse.mybir as mybir
from concourse.bass_utils import run_bass_kernel_spmd

F32 = mybir.dt.float32
F32R = mybir.dt.float32r
BF16 = mybir.dt.bfloat16
FP8 = mybir.dt.float8e4

N_CORES = 8
B, TQ, TC, H = 16, 2048, 2048, 1024

PV_FP8 = False         # fp8 PV fails the 2e-2 error gate; keep bf16


def build_bass(b_loc, tq, tc, h, n_cores=N_CORES):
    """Build the per-core Bass graph. All cores run the same graph (SPMD)."""
    d = 2 * h
    ho = h
    n_qt = tq // 128       # q tiles
    n_kt = tc // 128       # k tiles
    n_hc = h // 128        # h chunks
    n_dc = d // 128        # d chunks (contraction for proj)
    kb = min(512, tc)      # QK rhs block (fp32 moving-operand max)
    n_kb = tc // kb
    hob = min(512, ho)     # proj output block
    n_hob = ho // hob
    SUPER = 2              # q-tiles per super-iteration
    assert n_qt % SUPER == 0
    n_s = n_qt // SUPER
    sq = SUPER * 128       # q columns per super-iteration
    qg = min(4, n_hc)      # f32 transposes packed per PSUM bank
    pg = min(8, n_kt)      # bf16 transposes packed per PSUM bank

    nc = bacc.Bacc("TRN2", target_bir_lowering=False, debug=False,
                   num_devices=n_cores)

    q_ext = nc.declare_dram_parameter("q", [b_loc, tq, h], F32R, isOutput=False)
    c_ext = nc.declare_dram_parameter("c", [b_loc, tc, h], F32R, isOutput=False)
    wt_ext = nc.declare_dram_parameter("wt", [d, ho], F32, isOutput=False)
    idf_ext = nc.declare_dram_parameter("idf", [128, 128], F32, isOutput=False)
    idr_ext = nc.declare_dram_parameter("idr", [128, 128], F32R, isOutput=False)
    idb_ext = nc.declare_dram_parameter("idb", [128, 128], BF16, isOutput=False)
    ones_ext = nc.declare_dram_parameter("ones", [1, 128], F32R, isOutput=False)
    out_ext = nc.declare_dram_parameter("out", [b_loc, tq, ho], F32, isOutput=True)

    with tile.TileContext(nc) as tc_:
        with (
            tc_.tile_pool(name="const", bufs=1) as const_pool,
            tc_.tile_pool(name="stage", bufs=4) as stage_pool,
            tc_.tile_pool(name="ct", bufs=1) as ct_pool,
            tc_.tile_pool(name="cbf", bufs=1) as cbf_pool,
            tc_.tile_pool(name="qt", bufs=2) as qt_pool,
            tc_.tile_pool(name="p", bufs=2 * SUPER) as p_pool,
            tc_.tile_pool(name="ptb", bufs=1) as pt_pool,
            tc_.tile_pool(name="comb", bufs=2) as comb_pool,
            tc_.tile_pool(name="ostage", bufs=2) as out_pool,
            tc_.tile_pool(name="stats", bufs=12) as stats_pool,
            tc_.tile_pool(name="rrow", bufs=2) as rrow_pool,
            tc_.tile_pool(name="rcpb", bufs=2) as rcpb_pool,
            tc_.tile_pool(name="ps_s", bufs=1, space="PSUM") as ps_s,
            tc_.tile_pool(name="ps_tp", bufs=2, space="PSUM") as ps_tp,
            tc_.tile_pool(name="ps_mm", bufs=2, space="PSUM") as ps_mm,
        ):
            p_dt = FP8 if PV_FP8 else BF16
            # --- constants: identities + W^T (bf16) + ones row ---
            idf = const_pool.tile([128, 128], F32, tag="idf")
            nc.sync.dma_start(idf[:], idf_ext[:])
            idr = const_pool.tile([128, 128], F32R, tag="idr")
            nc.sync.dma_start(idr[:], idr_ext[:])
            idb = const_pool.tile([128, 128], BF16, tag="idb")
            nc.sync.dma_start(idb[:], idb_ext[:])
            ones_r = const_pool.tile([1, 128], F32R, tag="ones")
            nc.sync.dma_start(ones_r[:], ones_ext[:])

            wt_bf = const_pool.tile([128, n_dc * ho], BF16, tag="wtbf")

            def emit_wt_setup():
                for dc in range(n_dc):
                    ws = stage_pool.tile([128, ho], F32, tag="stage",
                                         name=f"ws_{dc}")
                    nc.sync.dma_start(ws[:], wt_ext[dc * 128:(dc + 1) * 128, :])
                    if dc % 2 == 0:
                        nc.vector.tensor_copy(
                            wt_bf[:, dc * ho:(dc + 1) * ho], ws[:])
                    else:
                        nc.scalar.copy(wt_bf[:, dc * ho:(dc + 1) * ho], ws[:])

            p_tiles = {}      # (b, t) -> unnormalized quantized P tile
            rcp_tiles = {}    # (b, t) -> [128, 1] reciprocal row sums
            combT_map = {}    # s -> combT tile of current batch
            pt_map = {}       # s -> P^T tile of current batch
            rcpb_map = {}     # s -> [128, sq] broadcast reciprocal tile

            def emit_qtr(b, s, ti, qs=None):
                """Q load + QT transposes; returns qt_t for the QK stage."""
                t = s * SUPER + ti
                combT = combT_map[(b, s)]
                comb_r = combT.rearrange("p (dc q) -> p dc q", q=sq)
                if qs is None:
                    qs = stage_pool.tile([128, h], F32R, tag="stage",
                                         name=f"qs_{b}_{t}")
                    nc.sync.dma_start(qs[:], q_ext[b, t * 128:(t + 1) * 128, :])
                qt_t = qt_pool.tile([128, h], F32R, tag="qt",
                                    name=f"qt_{b}_{t}")
                for g in range(n_hc // qg):
                    tq4 = ps_tp.tile([128, qg * 128], F32R, tag="tp",
                                     name=f"tq4_{b}_{t}_{g}")
                    for j in range(qg):
                        hc = qg * g + j
                        nc.tensor.transpose(
                            tq4[:, j * 128:(j + 1) * 128],
                            qs[:, hc * 128:(hc + 1) * 128], idr[:])
                    dst = qt_t[:, g * qg * 128:(g + 1) * qg * 128]
                    if g % 2 == 0:
                        nc.scalar.copy(dst, tq4[:])
                    else:
                        nc.vector.tensor_copy(dst, tq4[:])
                nc.vector.tensor_copy(
                    comb_r[:, n_hc: 2 * n_hc, ti * 128:(ti + 1) * 128],
                    qt_t.rearrange("p (j c) -> p j c", c=128)[:])
                return qt_t

            def emit_qk_block(b, t, qt_t, ct_all, kbi, s_ps):
                """One kb-wide column block of the QK matmuls (hc sweep)."""
                for hc in range(n_hc):
                    lhs = qt_t[:, hc * 128:(hc + 1) * 128]
                    rhs = ct_all[:, hc * tc + kbi * kb:
                                 hc * tc + (kbi + 1) * kb]
                    nc.tensor.matmul(
                        s_ps[:, kbi * kb:(kbi + 1) * kb], lhs, rhs,
                        start=(hc == 0), stop=(hc == n_hc - 1))

            def emit_softmax(b, t, s_ps):
                """Softmax chain on a finished scores PSUM tile.

                exp output is the UNNORMALIZED quantized P (max ~= 1);
                the row-sum (of exact exp values) is accumulated into
                l_tot and its reciprocal kept for the PV-drain
                normalization."""
                negm = stats_pool.tile([128, 1], F32, tag="negm",
                                       name=f"negm_{b}_{t}")
                nc.vector.reduce_max(
                    negm[:], s_ps[:], axis=mybir.AxisListType.X, negate=True)
                l_tot = stats_pool.tile([128, 1], F32, tag="ltot",
                                        name=f"lt_{b}_{t}")
                nc.vector.memset(l_tot[:], 0.0)
                p = p_pool.tile([128, tc], BF16, tag="p", name=f"p_{b}_{t}")
                nc.scalar.activation(
                    p[:], s_ps[:], mybir.ActivationFunctionType.Exp,
                    bias=negm[:], scale=1.0, accum_out=l_tot[:])
                rcp = stats_pool.tile([128, 1], F32, tag="rcp",
                                      name=f"rcp_{b}_{t}")
                nc.vector.reciprocal(rcp[:], l_tot[:])
                p_tiles[(b, t)] = p
                rcp_tiles[(b, t)] = rcp

            def emit_qk_softmax(b, s, ti, qt_t, ct_all):
                t = s * SUPER + ti
                s_ps = ps_s.tile([128, tc], F32, tag="s", name=f"s_{b}_{t}")
                for hc in range(n_hc):
                    for kbi in range(n_kb):
                        lhs = qt_t[:, hc * 128:(hc + 1) * 128]
                        rhs = ct_all[:, hc * tc + kbi * kb:
                                     hc * tc + (kbi + 1) * kb]
                        nc.tensor.matmul(
                            s_ps[:, kbi * kb:(kbi + 1) * kb], lhs, rhs,
                            start=(hc == 0), stop=(hc == n_hc - 1))
                emit_softmax(b, t, s_ps)

            def emit_rcpb(b, s):
                """[128, sq] broadcast of 1/rowsum for super s: PE
                transpose each tile's rcp [128,1] into a shared row,
                then ones-outer-product to broadcast across partitions."""
                row_ps = ps_tp.tile([128, qg * 128], F32, tag="tp",
                                    name=f"rrow_{b}_{s}")
                for ti in range(SUPER):
                    rcp = rcp_tiles.pop((b, s * SUPER + ti))
                    nc.tensor.transpose(
                        row_ps[0:1, ti * 128:(ti + 1) * 128], rcp[:], idf[:])
                row_sb = rrow_pool.tile([1, sq], F32R, tag="rrow",
                                        name=f"rrs_{b}_{s}")
                nc.scalar.copy(row_sb[:], row_ps[0:1, 0:sq])
                bc_ps = ps_mm.tile([128, sq], F32, tag="mm",
                                   name=f"rbc_{b}_{s}")
                nc.tensor.matmul(bc_ps[:], ones_r[:], row_sb[:],
                                 start=True, stop=True)
                rcpb = rcpb_pool.tile([128, sq], F32, tag="rcpb",
                                      name=f"rcpb_{b}_{s}")
                nc.vector.tensor_copy(rcpb[:], bc_ps[:])
                rcpb_map[(b, s)] = rcpb

            def emit_pt(b, s):
                """P^T for super s: bf16 PE transposes packed into PSUM
                banks, drained by wide ACT/DVE copies that cast to the
                PV dtype (fp8 when PV_FP8)."""
                pt_big = pt_pool.tile([128, n_kt * sq], p_dt, tag="ptb",
                                      name=f"ptb_{b}_{s}")
                pt_r = pt_big.rearrange("p (k q) -> p k q", q=sq)
                for ti in range(SUPER):
                    p = p_tiles.pop((b, s * SUPER + ti))
                    for g in range(n_kt // pg):
                        tp8 = ps_tp.tile([128, pg * 128], BF16, tag="tp",
                                         name=f"tp8_{b}_{s}_{ti}_{g}")
                        for j in range(pg):
                            kt = pg * g + j
                            nc.tensor.transpose(
                                tp8[:, j * 128:(j + 1) * 128],
                                p[:, kt * 128:(kt + 1) * 128], idb[:])
                        if (ti + g) % 2 == 0:
                            nc.scalar.copy(
                                pt_r[:, pg * g: pg * (g + 1),
                                     ti * 128:(ti + 1) * 128],
                                tp8.rearrange("p (j c) -> p j c", c=128)[:])
                        else:
                            nc.vector.tensor_copy(
                                pt_r[:, pg * g: pg * (g + 1),
                                     ti * 128:(ti + 1) * 128],
                                tp8.rearrange("p (j c) -> p j c", c=128)[:])
                pt_map[(b, s)] = pt_big

            def emit_pv(b, s, c_q):
                """PV matmuls: mix^T chunks into combT for super s.

                fp8 path: DoubleRow pairs adjacent k-tiles (2x rate).
                Drain multiplies by the rcpb broadcast (renormalize)."""
                combT = combT_map[(b, s)]
                pt_big = pt_map.pop((b, s))
                rcpb = rcpb_map.pop((b, s))
                c_r = c_q.rearrange("p (k h2) -> p k h2", h2=h)
                pt_r2 = pt_big.rearrange("p (k q) -> p k q", q=sq)
                for hc in range(n_hc):
                    mm = ps_mm.tile([128, sq], F32, tag="mm",
                                    name=f"mm_{b}_{s}_{hc}")
                    if PV_FP8:
                        for kt in range(0, n_kt, 2):
                            nc.tensor.matmul(
                                mm[:],
                                c_r[:, kt:kt + 2, hc * 128:(hc + 1) * 128],
                                pt_r2[:, kt:kt + 2, :],
                                start=(kt == 0), stop=(kt == n_kt - 2),
                                perf_mode=mybir.MatmulPerfMode.DoubleRow)
                    else:
                        for kt in range(n_kt):
                            nc.tensor.matmul(
                                mm[:],
                                c_q[:, kt * h + hc * 128: kt * h + (hc + 1) * 128],
                                pt_big[:, kt * sq:(kt + 1) * sq],
                                start=(kt == 0), stop=(kt == n_kt - 1))
                    nc.vector.tensor_mul(
                        combT[:, hc * sq:(hc + 1) * sq], mm[:], rcpb[:])

            def emit_proj(b, s):
                """Projection + tanh + store for both tiles of super s."""
                combT = combT_map.pop((b, s))
                for ti in range(SUPER):
                    t = s * SUPER + ti
                    ostage = out_pool.tile([128, ho], F32, tag="ostage",
                                           name=f"os_{b}_{t}")
                    for hb in range(n_hob):
                        pr = ps_mm.tile([128, hob], F32, tag="mm",
                                        name=f"pr_{b}_{t}_{hb}")
                        for dc in range(n_dc):
                            nc.tensor.matmul(
                                pr[:],
                                combT[:, dc * sq + ti * 128:
                                      dc * sq + (ti + 1) * 128],
                                wt_bf[:, dc * ho + hb * hob:
                                      dc * ho + (hb + 1) * hob],
                                start=(dc == 0), stop=(dc == n_dc - 1))
                        nc.scalar.activation(
                            ostage[:, hb * hob:(hb + 1) * hob], pr[:],
                            mybir.ActivationFunctionType.Tanh)
                    nc.sync.dma_start(
                        out_ext[b, t * 128:(t + 1) * 128, :], ostage[:])

            for b in range(b_loc):
                # prefetch the first super's Q tiles ahead of the C DMAs so
                # the first Qtr transposes are not stuck behind 16 MB of C/W
                q_pre = []
                for ti in range(SUPER):
                    qp = stage_pool.tile([128, h], F32R, tag="stage",
                                         name=f"qpre_{b}_{ti}")
                    nc.sync.dma_start(qp[:], q_ext[b, ti * 128:(ti + 1) * 128, :])
                    q_pre.append(qp)
                # --- batch setup: CT (f32r, [h, k]) and C (fp8/bf16, [k, h]) ---
                ct_all = ct_pool.tile([128, n_hc * tc], F32R, tag="ct",
                                      name=f"ct_{b}")
                ct_r = ct_all.rearrange("p (hc k) -> p hc k", k=tc)
                c_q = cbf_pool.tile([128, n_kt * h], p_dt, tag="cbf",
                                    name=f"cbf_{b}")

                def emit_c_setup(kt):
                    cs = stage_pool.tile([128, h], F32R, tag="stage",
                                         name=f"cs_{b}_{kt}")
                    nc.sync.dma_start(cs[:], c_ext[b, kt * 128:(kt + 1) * 128, :])
                    if kt % 2 == 0:
                        nc.vector.tensor_copy(
                            c_q[:, kt * h:(kt + 1) * h], cs[:])
                    else:
                        nc.scalar.copy(c_q[:, kt * h:(kt + 1) * h], cs[:])
                    for g in range(n_hc // qg):
                        tc4 = ps_tp.tile([128, qg * 128], F32R, tag="tp",
                                         name=f"tc4_{b}_{kt}_{g}")
                        for j in range(qg):
                            hc = qg * g + j
                            nc.tensor.transpose(
                                tc4[:, j * 128:(j + 1) * 128],
                                cs[:, hc * 128:(hc + 1) * 128], idr[:])
                        dst = ct_r[:, qg * g: qg * (g + 1),
                                   kt * 128:(kt + 1) * 128]
                        src = tc4.rearrange("p (j c) -> p j c", c=128)[:]
                        if (g + kt) % 2 == 1:
                            nc.scalar.copy(dst, src)
                        else:
                            nc.vector.tensor_copy(dst, src)

                # C setup interleaved with the first q-tile's QK blocks:
                # each kb-wide score block starts as soon as its 4 k-tiles
                # of CT are transposed, so the PE is not DMA-paced
                combT_map[(b, 0)] = comb_pool.tile(
                    [128, n_dc * sq], BF16, tag="comb", name=f"cb_{b}_0")
                ktpb = kb // 128   # k-tiles per score block
                qt0_first = None
                s_ps0 = None
                for kbi in range(n_kb):
                    for kt in range(ktpb * kbi, ktpb * (kbi + 1)):
                        emit_c_setup(kt)
                    if kbi == 0:
                        qt0_first = emit_qtr(b, 0, 0, qs=q_pre[0])
                        s_ps0 = ps_s.tile([128, tc], F32, tag="s",
                                          name=f"s_{b}_0")
                    emit_qk_block(b, 0, qt0_first, ct_all, kbi, s_ps0)
                if b == 0:
                    emit_wt_setup()
                emit_softmax(b, 0, s_ps0)

                # --- pipelined main loop ---
                for s in range(n_s):
                    if s > 0:
                        emit_rcpb(b, s - 1)
                        combT_map[(b, s)] = comb_pool.tile(
                            [128, n_dc * sq], BF16, tag="comb",
                            name=f"cb_{b}_{s}")
                        qt0 = emit_qtr(b, s, 0)
                        emit_pt(b, s - 1)
                        emit_qk_softmax(b, s, 0, qt0, ct_all)
                    qt1 = emit_qtr(b, s, 1, qs=q_pre[1] if s == 0 else None)
                    if s > 0:
                        emit_pv(b, s - 1, c_q)
                    emit_qk_softmax(b, s, 1, qt1, ct_all)
                    if s > 0:
                        emit_proj(b, s - 1)
                emit_rcpb(b, n_s - 1)
                emit_pt(b, n_s - 1)
                emit_pv(b, n_s - 1, c_q)
                emit_proj(b, n_s - 1)

    nc.compile()
    return nc


_NC_CACHE = {}


def _get_nc(b_loc, tq, tc, h):
    key = (b_loc, tq, tc, h)
    if key not in _NC_CACHE:
        _NC_CACHE[key] = build_bass(b_loc, tq, tc, h)
    return _NC_CACHE[key]


def make_in_maps(query, context, W_attn, n_cores=N_CORES):
    b = query.shape[0]
    b_loc = b // n_cores
    wt = np.ascontiguousarray(W_attn.T.astype(np.float32))
    idf = np.eye(128, dtype=np.float32)
    idb = np.eye(128).astype(ml_dtypes.bfloat16)
    in_maps = []
    for i in range(n_cores):
        in_maps.append({
            "q": np.ascontiguousarray(
                query[i * b_loc:(i + 1) * b_loc].astype(np.float32)),
            "c": np.ascontiguousarray(
                context[i * b_loc:(i + 1) * b_loc].astype(np.float32)),
            "wt": wt,
            "idf": idf,
            "idr": idf,
            "idb": idb,
            "ones": np.ones((1, 128), dtype=np.float32),
        })
    return in_maps


def kernel(query, context, W_attn, _trace=False, _trace_kwargs=None):
    b, tq, h = query.shape
    tc = context.shape[1]
    b_loc = b // N_CORES
    nc = _get_nc(b_loc, tq, tc, h)
    in_maps = make_in_maps(query, context, W_attn)
    res = run_bass_kernel_spmd(
        nc, in_maps, core_ids=list(range(N_CORES)), trace=_trace,
        **(_trace_kwargs or {}))
    out = np.concatenate([res.results[i]["out"] for i in range(N_CORES)], axis=0)
    if _trace:
        return out, res
    return out


# revision 20
# speedup vs baseline: 1.0068x; 1.0068x over previous
"""Trainium2 8-core kernel for batched attention + concat projection.

Reference computation (per batch b):
    scores = Q @ C^T                  [TQ, TC]
    A      = softmax(scores, axis=-1)
    mix    = A @ C                    [TQ, H]
    out    = tanh(concat([mix, Q]) @ W^T)   [TQ, H]

Distribution: pure data-parallel over batch (B=16 across 8 cores, 2
batches per core), W replicated. No collectives needed.

Per-core dataflow (activations kept in "transposed" [feature, token]
layout so every matmul contracts over the partition axis):
  - CT = C^T (f32r) and QT = Q^T built on-device via PE transposes.
  - scores S[q,k] = QT.T @ CT  (f32r matmuls, 1 col/cycle).
  - softmax over free axis k: DVE reduce_max(negate) -> ACT exp with
    per-partition bias, bf16 output (unnormalized, max ~= 1) and
    fp32 row-sum accumulator -> DVE reciprocal.
  - P^T via bf16 PE transposes, mix^T = C.T @ P^T in bf16.
  - normalization folded into the PV PSUM drain: multiply by a
    [128, sq] broadcast of 1/rowsum built once per super-iteration on
    the PE (transpose rcp to a row + ones outer-product matmul).
  - proj: out[q, :] = tanh(combT.T @ W^T) in bf16, W^T pre-transposed
    on host.

The P^T/PV/proj stages for super-iteration s are emitted one
super-iteration later (software pipelining) so the in-order TensorE
stream always has ready matmul work while the softmax chain of the
current tile runs on ACT/DVE.
"""

import numpy as np
import ml_dtypes

import concourse.bacc as bacc
import concourse.tile as tile
import concourse.mybir as mybir
from concourse.bass_utils import run_bass_kernel_spmd

F32 = mybir.dt.float32
F32R = mybir.dt.float32r
BF16 = mybir.dt.bfloat16
FP8 = mybir.dt.float8e4

N_CORES = 8
B, TQ, TC, H = 16, 2048, 2048, 1024

PV_FP8 = False         # fp8 PV fails the 2e-2 error gate; keep bf16


def build_bass(b_loc, tq, tc, h, n_cores=N_CORES):
    """Build the per-core Bass graph. All cores run the same graph (SPMD)."""
    d = 2 * h
    ho = h
    n_qt = tq // 128       # q tiles
    n_kt = tc // 128       # k tiles
    n_hc = h // 128        # h chunks
    n_dc = d // 128        # d chunks (contraction for proj)
    kb = min(512, tc)      # QK rhs block (fp32 moving-operand max)
    n_kb = tc // kb
    hob = min(512, ho)     # proj output block
    n_hob = ho // hob
    SUPER = 2              # q-tiles per super-iteration
    assert n_qt % SUPER == 0
    n_s = n_qt // SUPER
    sq = SUPER * 128       # q columns per super-iteration
    qg = min(4, n_hc)      # f32 transposes packed per PSUM bank
    pg = min(8, n_kt)      # bf16 transposes packed per PSUM bank

    nc = bacc.Bacc("TRN2", target_bir_lowering=False, debug=False,
                   num_devices=n_cores)

    q_ext = nc.declare_dram_parameter("q", [b_loc, tq, h], F32R, isOutput=False)
    c_ext = nc.declare_dram_parameter("c", [b_loc, tc, h], F32R, isOutput=False)
    wt_ext = nc.declare_dram_parameter("wt", [d, ho], F32, isOutput=False)
    idf_ext = nc.declare_dram_parameter("idf", [128, 128], F32, isOutput=False)
    idr_ext = nc.declare_dram_parameter("idr", [128, 128], F32R, isOutput=False)
    idb_ext = nc.declare_dram_parameter("idb", [128, 128], BF16, isOutput=False)
    ones_ext = nc.declare_dram_parameter("ones", [1, 128], F32R, isOutput=False)
    out_ext = nc.declare_dram_parameter("out", [b_loc, tq, ho], F32, isOutput=True)

    with tile.TileContext(nc) as tc_:
        with (
            tc_.tile_pool(name="const", bufs=1) as const_pool,
            tc_.tile_pool(name="stage", bufs=4) as stage_pool,
            tc_.tile_pool(name="ct", bufs=1) as ct_pool,
            tc_.tile_pool(name="cbf", bufs=1) as cbf_pool,
            tc_.tile_pool(name="qt", bufs=2) as qt_pool,
            tc_.tile_pool(name="p", bufs=2 * SUPER) as p_pool,
            tc_.tile_pool(name="ptb", bufs=1) as pt_pool,
            tc_.tile_pool(name="comb", bufs=2) as comb_pool,
            tc_.tile_pool(name="ostage", bufs=2) as out_pool,
            tc_.tile_pool(name="stats", bufs=12) as stats_pool,
            tc_.tile_pool(name="rrow", bufs=2) as rrow_pool,
            tc_.tile_pool(name="rcpb", bufs=2) as rcpb_pool,
            tc_.tile_pool(name="ps_s", bufs=1, space="PSUM") as ps_s,
            tc_.tile_pool(name="ps_tp", bufs=2, space="PSUM") as ps_tp,
            tc_.tile_pool(name="ps_mm", bufs=2, space="PSUM") as ps_mm,
        ):
            p_dt = FP8 if PV_FP8 else BF16
            # --- constants: identities + W^T (bf16) + ones row ---
            idf = const_pool.tile([128, 128], F32, tag="idf")
            nc.sync.dma_start(idf[:], idf_ext[:])
            idr = const_pool.tile([128, 128], F32R, tag="idr")
            nc.sync.dma_start(idr[:], idr_ext[:])
            idb = const_pool.tile([128, 128], BF16, tag="idb")
            nc.sync.dma_start(idb[:], idb_ext[:])
            ones_r = const_pool.tile([1, 128], F32R, tag="ones")
            nc.sync.dma_start(ones_r[:], ones_ext[:])

            wt_bf = const_pool.tile([128, n_dc * ho], BF16, tag="wtbf")

            def emit_wt_chunk(phase):
                for dc in range(4 * phase, 4 * (phase + 1)):
                    ws = stage_pool.tile([128, ho], F32, tag="stage",
                                         name=f"ws_{dc}")
                    nc.sync.dma_start(ws[:], wt_ext[dc * 128:(dc + 1) * 128, :])
                    if dc % 2 == 0:
                        nc.vector.tensor_copy(
                            wt_bf[:, dc * ho:(dc + 1) * ho], ws[:])
                    else:
                        nc.scalar.copy(wt_bf[:, dc * ho:(dc + 1) * ho], ws[:])

            p_tiles = {}      # (b, t) -> unnormalized quantized P tile
            rcp_tiles = {}    # (b, t) -> [128, 1] reciprocal row sums
            combT_map = {}    # s -> combT tile of current batch
            pt_map = {}       # s -> P^T tile of current batch
            rcpb_map = {}     # s -> [128, sq] broadcast reciprocal tile

            def emit_qtr(b, s, ti, qs=None):
                """Q load + QT transposes; returns qt_t for the QK stage."""
                t = s * SUPER + ti
                combT = combT_map[(b, s)]
                comb_r = combT.rearrange("p (dc q) -> p dc q", q=sq)
                if qs is None:
                    qs = stage_pool.tile([128, h], F32R, tag="stage",
                                         name=f"qs_{b}_{t}")
                    nc.sync.dma_start(qs[:], q_ext[b, t * 128:(t + 1) * 128, :])
                qt_t = qt_pool.tile([128, h], F32R, tag="qt",
                                    name=f"qt_{b}_{t}")
                for g in range(n_hc // qg):
                    tq4 = ps_tp.tile([128, qg * 128], F32R, tag="tp",
                                     name=f"tq4_{b}_{t}_{g}")
                    for j in range(qg):
                        hc = qg * g + j
                        nc.tensor.transpose(
                            tq4[:, j * 128:(j + 1) * 128],
                            qs[:, hc * 128:(hc + 1) * 128], idr[:])
                    dst = qt_t[:, g * qg * 128:(g + 1) * qg * 128]
                    if g % 2 == 0:
                        nc.scalar.copy(dst, tq4[:])
                    else:
                        nc.vector.tensor_copy(dst, tq4[:])
                nc.vector.tensor_copy(
                    comb_r[:, n_hc: 2 * n_hc, ti * 128:(ti + 1) * 128],
                    qt_t.rearrange("p (j c) -> p j c", c=128)[:])
                return qt_t

            def emit_qk_block(b, t, qt_t, ct_all, kbi, s_ps):
                """One kb-wide column block of the QK matmuls (hc sweep)."""
                for hc in range(n_hc):
                    lhs = qt_t[:, hc * 128:(hc + 1) * 128]
                    rhs = ct_all[:, hc * tc + kbi * kb:
                                 hc * tc + (kbi + 1) * kb]
                    nc.tensor.matmul(
                        s_ps[:, kbi * kb:(kbi + 1) * kb], lhs, rhs,
                        start=(hc == 0), stop=(hc == n_hc - 1))

            def emit_softmax(b, t, s_ps):
                """Softmax chain on a finished scores PSUM tile.

                exp output is the UNNORMALIZED quantized P (max ~= 1);
                the row-sum (of exact exp values) is accumulated into
                l_tot and its reciprocal kept for the PV-drain
                normalization."""
                negm = stats_pool.tile([128, 1], F32, tag="negm",
                                       name=f"negm_{b}_{t}")
                nc.vector.reduce_max(
                    negm[:], s_ps[:], axis=mybir.AxisListType.X, negate=True)
                l_tot = stats_pool.tile([128, 1], F32, tag="ltot",
                                        name=f"lt_{b}_{t}")
                nc.vector.memset(l_tot[:], 0.0)
                p = p_pool.tile([128, tc], BF16, tag="p", name=f"p_{b}_{t}")
                nc.scalar.activation(
                    p[:], s_ps[:], mybir.ActivationFunctionType.Exp,
                    bias=negm[:], scale=1.0, accum_out=l_tot[:])
                rcp = stats_pool.tile([128, 1], F32, tag="rcp",
                                      name=f"rcp_{b}_{t}")
                nc.vector.reciprocal(rcp[:], l_tot[:])
                p_tiles[(b, t)] = p
                rcp_tiles[(b, t)] = rcp

            def emit_qk_softmax(b, s, ti, qt_t, ct_all):
                t = s * SUPER + ti
                s_ps = ps_s.tile([128, tc], F32, tag="s", name=f"s_{b}_{t}")
                for hc in range(n_hc):
                    for kbi in range(n_kb):
                        lhs = qt_t[:, hc * 128:(hc + 1) * 128]
                        rhs = ct_all[:, hc * tc + kbi * kb:
                                     hc * tc + (kbi + 1) * kb]
                        nc.tensor.matmul(
                            s_ps[:, kbi * kb:(kbi + 1) * kb], lhs, rhs,
                            start=(hc == 0), stop=(hc == n_hc - 1))
                emit_softmax(b, t, s_ps)

            def emit_rcpb_row(b, s):
                """Transpose the two rcp [128,1] columns into one row."""
                row_ps = ps_tp.tile([128, qg * 128], F32, tag="tp",
                                    name=f"rrow_{b}_{s}")
                for ti in range(SUPER):
                    rcp = rcp_tiles.pop((b, s * SUPER + ti))
                    nc.tensor.transpose(
                        row_ps[0:1, ti * 128:(ti + 1) * 128], rcp[:], idf[:])
                row_sb = rrow_pool.tile([1, sq], F32R, tag="rrow",
                                        name=f"rrs_{b}_{s}")
                nc.scalar.copy(row_sb[:], row_ps[0:1, 0:sq])
                return row_sb

            def emit_rcpb_bcast(b, s, row_sb):
                """Ones outer-product broadcast of 1/rowsum to [128, sq]."""
                bc_ps = ps_mm.tile([128, sq], F32, tag="mm",
                                   name=f"rbc_{b}_{s}")
                nc.tensor.matmul(bc_ps[:], ones_r[:], row_sb[:],
                                 start=True, stop=True)
                rcpb = rcpb_pool.tile([128, sq], F32, tag="rcpb",
                                      name=f"rcpb_{b}_{s}")
                nc.vector.tensor_copy(rcpb[:], bc_ps[:])
                rcpb_map[(b, s)] = rcpb

            def emit_pt(b, s):
                """P^T for super s: bf16 PE transposes packed into PSUM
                banks, drained by wide ACT/DVE copies that cast to the
                PV dtype (fp8 when PV_FP8)."""
                pt_big = pt_pool.tile([128, n_kt * sq], p_dt, tag="ptb",
                                      name=f"ptb_{b}_{s}")
                pt_r = pt_big.rearrange("p (k q) -> p k q", q=sq)
                ps = [p_tiles.pop((b, s * SUPER + ti)) for ti in range(SUPER)]
                for g in range(n_kt // pg):
                    for ti in range(SUPER):
                        tp8 = ps_tp.tile([128, pg * 128], BF16, tag="tp",
                                         name=f"tp8_{b}_{s}_{ti}_{g}")
                        for j in range(pg):
                            kt = pg * g + j
                            nc.tensor.transpose(
                                tp8[:, j * 128:(j + 1) * 128],
                                ps[ti][:, kt * 128:(kt + 1) * 128], idb[:])
                        if ti % 2 == 0:
                            nc.scalar.copy(
                                pt_r[:, pg * g: pg * (g + 1),
                                     ti * 128:(ti + 1) * 128],
                                tp8.rearrange("p (j c) -> p j c", c=128)[:])
                        else:
                            nc.vector.tensor_copy(
                                pt_r[:, pg * g: pg * (g + 1),
                                     ti * 128:(ti + 1) * 128],
                                tp8.rearrange("p (j c) -> p j c", c=128)[:])
                pt_map[(b, s)] = pt_big

            def emit_pv(b, s, c_q):
                """PV matmuls: mix^T chunks into combT for super s.

                fp8 path: DoubleRow pairs adjacent k-tiles (2x rate).
                Drain multiplies by the rcpb broadcast (renormalize)."""
                combT = combT_map[(b, s)]
                pt_big = pt_map.pop((b, s))
                rcpb = rcpb_map.pop((b, s))
                c_r = c_q.rearrange("p (k h2) -> p k h2", h2=h)
                pt_r2 = pt_big.rearrange("p (k q) -> p k q", q=sq)
                for hc in range(n_hc):
                    mm = ps_mm.tile([128, sq], F32, tag="mm",
                                    name=f"mm_{b}_{s}_{hc}")
                    if PV_FP8:
                        for kt in range(0, n_kt, 2):
                            nc.tensor.matmul(
                                mm[:],
                                c_r[:, kt:kt + 2, hc * 128:(hc + 1) * 128],
                                pt_r2[:, kt:kt + 2, :],
                                start=(kt == 0), stop=(kt == n_kt - 2),
                                perf_mode=mybir.MatmulPerfMode.DoubleRow)
                    else:
                        for kt in range(n_kt):
                            nc.tensor.matmul(
                                mm[:],
                                c_q[:, kt * h + hc * 128: kt * h + (hc + 1) * 128],
                                pt_big[:, kt * sq:(kt + 1) * sq],
                                start=(kt == 0), stop=(kt == n_kt - 1))
                    nc.vector.tensor_mul(
                        combT[:, hc * sq:(hc + 1) * sq], mm[:], rcpb[:])

            def emit_proj(b, s):
                """Projection + tanh + store for both tiles of super s."""
                combT = combT_map.pop((b, s))
                for ti in range(SUPER):
                    t = s * SUPER + ti
                    ostage = out_pool.tile([128, ho], F32, tag="ostage",
                                           name=f"os_{b}_{t}")
                    for hb in range(n_hob):
                        pr = ps_mm.tile([128, hob], F32, tag="mm",
                                        name=f"pr_{b}_{t}_{hb}")
                        for dc in range(n_dc):
                            nc.tensor.matmul(
                                pr[:],
                                combT[:, dc * sq + ti * 128:
                                      dc * sq + (ti + 1) * 128],
                                wt_bf[:, dc * ho + hb * hob:
                                      dc * ho + (hb + 1) * hob],
                                start=(dc == 0), stop=(dc == n_dc - 1))
                        nc.scalar.activation(
                            ostage[:, hb * hob:(hb + 1) * hob], pr[:],
                            mybir.ActivationFunctionType.Tanh)
                    nc.sync.dma_start(
                        out_ext[b, t * 128:(t + 1) * 128, :], ostage[:])

            q_pre_map = {}
            cs_pre_map = {}
            for b in range(b_loc):
                # prefetch the first super's Q tiles ahead of the C DMAs so
                # the first Qtr transposes are not stuck behind 16 MB of C/W
                q_pre = q_pre_map.pop(b, None)
                if q_pre is None:
                    q_pre = []
                    for ti in range(SUPER):
                        qp = stage_pool.tile([128, h], F32R, tag="stage",
                                             name=f"qpre_{b}_{ti}")
                        nc.sync.dma_start(qp[:],
                                          q_ext[b, ti * 128:(ti + 1) * 128, :])
                        q_pre.append(qp)
                # --- batch setup: CT (f32r, [h, k]) and C (bf16, [k, h]) ---
                ct_all = ct_pool.tile([128, n_hc * tc], F32R, tag="ct",
                                      name=f"ct_{b}")
                ct_r = ct_all.rearrange("p (hc k) -> p hc k", k=tc)
                c_q = cbf_pool.tile([128, n_kt * h], p_dt, tag="cbf",
                                    name=f"cbf_{b}")

                def emit_cs_dma(bb, kt):
                    cs = stage_pool.tile([128, h], F32R, tag="stage",
                                         name=f"cs_{bb}_{kt}")
                    nc.sync.dma_start(cs[:],
                                      c_ext[bb, kt * 128:(kt + 1) * 128, :])
                    return cs

                def emit_c_setup(kt):
                    cs = cs_pre_map.pop((b, kt), None)
                    if cs is None:
                        cs = emit_cs_dma(b, kt)
                    if kt % 2 == 0:
                        nc.vector.tensor_copy(
                            c_q[:, kt * h:(kt + 1) * h], cs[:])
                    else:
                        nc.scalar.copy(c_q[:, kt * h:(kt + 1) * h], cs[:])
                    for g in range(n_hc // qg):
                        tc4 = ps_tp.tile([128, qg * 128], F32R, tag="tp",
                                         name=f"tc4_{b}_{kt}_{g}")
                        for j in range(qg):
                            hc = qg * g + j
                            nc.tensor.transpose(
                                tc4[:, j * 128:(j + 1) * 128],
                                cs[:, hc * 128:(hc + 1) * 128], idr[:])
                        dst = ct_r[:, qg * g: qg * (g + 1),
                                   kt * 128:(kt + 1) * 128]
                        src = tc4.rearrange("p (j c) -> p j c", c=128)[:]
                        if (g + kt) % 2 == 1:
                            nc.scalar.copy(dst, src)
                        else:
                            nc.vector.tensor_copy(dst, src)

                # first half of C, then the first Q-transpose (fills the
                # DMA-paced window), then the rest of C
                for kt in range(n_kt // 2):
                    emit_c_setup(kt)
                combT_map[(b, 0)] = comb_pool.tile(
                    [128, n_dc * sq], BF16, tag="comb", name=f"cb_{b}_0")
                qt0_first = emit_qtr(b, 0, 0, qs=q_pre[0])
                for kt in range(n_kt // 2, n_kt):
                    emit_c_setup(kt)
                if b == 0:
                    for ph in range(4):
                        emit_wt_chunk(ph)

                # --- pipelined main loop ---
                for s in range(n_s):
                    if s > 0:
                        row_sb = emit_rcpb_row(b, s - 1)
                        combT_map[(b, s)] = comb_pool.tile(
                            [128, n_dc * sq], BF16, tag="comb",
                            name=f"cb_{b}_{s}")
                        qt0 = emit_qtr(b, s, 0)
                        emit_rcpb_bcast(b, s - 1, row_sb)
                        emit_pt(b, s - 1)
                    else:
                        qt0 = qt0_first
                    emit_qk_softmax(b, s, 0, qt0, ct_all)
                    qt1 = emit_qtr(b, s, 1, qs=q_pre[1] if s == 0 else None)
                    if s > 0:
                        emit_pv(b, s - 1, c_q)
                    emit_qk_softmax(b, s, 1, qt1, ct_all)
                    if s > 0:
                        emit_proj(b, s - 1)
                row_sb = emit_rcpb_row(b, n_s - 1)
                emit_rcpb_bcast(b, n_s - 1, row_sb)
                emit_pt(b, n_s - 1)
                # prefetch the next batch's first C tiles + Q during the tail
                if b + 1 < b_loc:
                    qp2 = []
                    for ti in range(SUPER):
                        qp = stage_pool.tile([128, h], F32R, tag="stage",
                                             name=f"qpre_{b + 1}_{ti}")
                        nc.sync.dma_start(
                            qp[:], q_ext[b + 1, ti * 128:(ti + 1) * 128, :])
                        qp2.append(qp)
                    q_pre_map[b + 1] = qp2
                    for kt in range(2):
                        cs_pre_map[(b + 1, kt)] = emit_cs_dma(b + 1, kt)
                emit_pv(b, n_s - 1, c_q)
                if b + 1 < b_loc:
                    for kt in range(2, 4):
                        cs_pre_map[(b + 1, kt)] = emit_cs_dma(b + 1, kt)
                emit_proj(b, n_s - 1)

    nc.compile()
    return nc


_NC_CACHE = {}


def _get_nc(b_loc, tq, tc, h):
    key = (b_loc, tq, tc, h)
    if key not in _NC_CACHE:
        _NC_CACHE[key] = build_bass(b_loc, tq, tc, h)
    return _NC_CACHE[key]


def make_in_maps(query, context, W_attn, n_cores=N_CORES):
    b = query.shape[0]
    b_loc = b // n_cores
    wt = np.ascontiguousarray(W_attn.T.astype(np.float32))
    idf = np.eye(128, dtype=np.float32)
    idb = np.eye(128).astype(ml_dtypes.bfloat16)
    in_maps = []
    for i in range(n_cores):
        in_maps.append({
            "q": np.ascontiguousarray(
                query[i * b_loc:(i + 1) * b_loc].astype(np.float32)),
            "c": np.ascontiguousarray(
                context[i * b_loc:(i + 1) * b_loc].astype(np.float32)),
            "wt": wt,
            "idf": idf,
            "idr": idf,
            "idb": idb,
            "ones": np.ones((1, 128), dtype=np.float32),
        })
    return in_maps


def kernel(query, context, W_attn, _trace=False, _trace_kwargs=None):
    b, tq, h = query.shape
    tc = context.shape[1]
    b_loc = b // N_CORES
    nc = _get_nc(b_loc, tq, tc, h)
    in_maps = make_in_maps(query, context, W_attn)
    res = run_bass_kernel_spmd(
        nc, in_maps, core_ids=list(range(N_CORES)), trace=_trace,
        **(_trace_kwargs or {}))
    out = np.concatenate([res.results[i]["out"] for i in range(N_CORES)], axis=0)
    if _trace:
        return out, res
    return out
